# revision 1
# baseline (speedup 1.0000x reference)
"""Trainium2 Bass kernel for nn_Blur: depthwise 4x4 FIR blur (upfirdn2d pad=(2,1)).

Full inputs in, full output out. Internally shards the 4096 (b,c) images
across 8 NeuronCores (pure data parallel, no collectives).

Algorithm (per core, 512 images of [H=128, W=128]):
  out[ho, wo] = sum_{i,j} wf[i, j] * x[ho+i-2, wo+j-2]   (wf = flipped kernel)
which factors into 4 column-convolutions along H, each a banded matmul with
the contraction over the partition (H) axis, with the W-shift (j-2) realized
as a shifted PSUM write of an accumulating matmul:
  psum[:, c] += W_j^T @ x[:, c + (j-2)]     W_j[hi, ho] = wf[hi-ho+2, j]
Images are packed 3-per-PSUM-bank with 2-column zero gaps so the shifted
reads pick up zero padding at image edges and the moving free dim (391)
stays >= 256, where float32r matmuls run at 1 cycle/row.
"""

import os
import sys
from contextlib import ExitStack

for _p in ("/opt/trn_rl_repo", "/root/.axon_site/_ro/trn_rl_repo"):
    if os.path.isdir(_p) and _p not in sys.path:
        sys.path.append(_p)

import numpy as np

import concourse.bass as bass  # noqa: F401  (engine types referenced via nc)
import concourse.tile as tile
from concourse import bacc, bass_utils, mybir

B, C, H, W = 16, 256, 128, 128
N_CORES = 8
GROUP = 3          # images per PSUM bank / matmul group
STRIDE = 130       # 2-col gap + 128 data cols per image in the packed tile
PAD0 = 2           # upfirdn2d pad before (both spatial dims)

_PROGRAM_CACHE: dict[int, object] = {}


def _round_fp32r(a: np.ndarray) -> np.ndarray:
    """Round fp32 to fp32r (11-bit mantissa, RNE) — matches neuronxcc's
    static_cast_fp32_to_fp32r bit-exactly."""
    u = np.ascontiguousarray(a, dtype=np.float32).view(np.uint32)
    r = (u.astype(np.uint64) + 0x800 - ((u >> 12) & 1)) & 0xFFFFF000
    return r.astype(np.uint32).view(np.float32)


def _band_matrices(kern: np.ndarray) -> np.ndarray:
    """bands[j][hi, ho] = wf[hi-ho+2, j], wf = flip(kern). Shape [4,128,128]."""
    wf = np.flip(np.asarray(kern, dtype=np.float64), (0, 1))
    bands = np.zeros((4, H, H), dtype=np.float64)
    ho = np.arange(H)
    for j in range(4):
        for i in range(4):
            d = i - PAD0            # hi - ho
            hi = ho + d
            m = (hi >= 0) & (hi < H)
            bands[j][hi[m], ho[m]] = wf[i, j]
    return np.ascontiguousarray(bands.astype(np.float32))


def _groups(n_images: int):
    out = []
    i = 0
    while i < n_images:
        n = min(GROUP, n_images - i)
        out.append((i, n))
        i += n
    # avoid a trailing 1-image group (moving dim 131 < 256 is 4x slower):
    # rebalance the last two groups 3+1 -> 2+2
    if len(out) >= 2 and out[-1][1] == 1:
        i0, n0 = out[-2]
        out[-2] = (i0, 2)
        out[-1] = (i0 + 2, 2)
    return out


def build_program(n_images: int, xt_bufs: int = 12, qg: int = 4):
    """Build + compile the per-core Bass program for n_images [128,128] images.

    qg = PSUM banks in flight per weight-batched supergroup (j-outer order
    keeps PE matmuls dense); DMAs round-robin across engine DGE rings.
    """
    nc = bacc.Bacc("TRN2", target_bir_lowering=False, debug=False)
    f32 = mybir.dt.float32
    f32r = mybir.dt.float32r

    x_d = nc.dram_tensor("x", [n_images, H, W], f32r, kind="ExternalInput")
    b_d = nc.dram_tensor("bands", [4, H, H], f32r, kind="ExternalInput")
    z_d = nc.dram_tensor("zeros", [H, 2 * GROUP + 2], f32r, kind="ExternalInput")
    y_d = nc.dram_tensor("y", [n_images, H, W], f32, kind="ExternalOutput")

    wtot3 = STRIDE * GROUP + 2  # even width; cols {130k, 130k+1} are zero gaps

    with ExitStack() as ctx:
        tc = ctx.enter_context(tile.TileContext(nc))
        wpool = ctx.enter_context(tc.tile_pool(name="wpool", bufs=1))
        xpool = ctx.enter_context(tc.tile_pool(name="xpool", bufs=1))
        opool = ctx.enter_context(tc.tile_pool(name="opool", bufs=8))
        ppool = ctx.enter_context(tc.tile_pool(name="ppool", bufs=8, space="PSUM"))

        dma_engines = [nc.sync, nc.gpsimd]

        wt = wpool.tile([H, 4 * H], f32r)
        nc.sync.dma_start(
            wt.rearrange("p (j b) -> p j b", b=H), b_d.rearrange("j a b -> a j b")
        )

        # Persistent input tiles: gap columns are zeroed ONCE via DMA from the
        # zeros input; per-group DMAs only ever write data columns, so the
        # zero padding between images survives tile reuse. (A memset would be
        # simpler but fp32r memset fails walrus codegen / crashes the engine.)
        xts = []
        for k in range(xt_bufs):
            xt = xpool.tile([H, wtot3], f32r, name=f"xt{k}", tag=f"xt{k}")
            gaps = xt[:, 0 : STRIDE * GROUP].rearrange("p (k c) -> p k c", c=STRIDE)
            dma_engines[k % len(dma_engines)].dma_start(
                gaps[:, :, 0:PAD0],
                z_d[:, 0 : 2 * GROUP].rearrange("p (k c) -> p k c", c=PAD0),
            )
            dma_engines[(k + 1) % len(dma_engines)].dma_start(
                xt[:, STRIDE * GROUP : wtot3], z_d[:, 6:8]
            )
            xts.append(xt)

        gs = _groups(n_images)
        chunks = [gs[s : s + qg] for s in range(0, len(gs), qg)]

        def emit_in_dmas(ci):
            for q, (i0, n) in enumerate(chunks[ci]):
                g = ci * qg + q
                xt = xts[g % xt_bufs]
                # one batched input DMA per group
                dst = xt[:, 0 : STRIDE * n].rearrange("p (k c) -> p k c", c=STRIDE)
                dma_engines[g % len(dma_engines)].dma_start(
                    dst[:, :, PAD0 : PAD0 + W],
                    x_d[i0 : i0 + n].rearrange("n h w -> h n w"),
                )

        emit_in_dmas(0)
        if len(chunks) > 1:
            emit_in_dmas(1)
        for ci, chunk in enumerate(chunks):
            # software-pipelined emission: prefetch two supergroups ahead so
            # PE never starves (HAM stays warm). xt_bufs = 3*qg keeps the
            # in-flight tiles distinct.
            if ci + 2 < len(chunks):
                emit_in_dmas(ci + 2)

            pts = [
                ppool.tile([H, STRIDE * n + 2], f32, tag="pt", name="pt")
                for (i0, n) in chunk
            ]
            # fp32r matmul ISA restrictions: dst start col even (8B-aligned)
            # and dst/src innermost lengths even. dst [2, 130n+2) (j<=2) /
            # [2, 130n) (j=3); extra columns land in zero-gap / never-read
            # psum columns, so correctness is unaffected. j-outer order so
            # the 4 weight loads amortize over qg matmuls each.
            for idx, j in enumerate((2, 0, 1, 3)):  # full-width write first
                d = j - PAD0
                for q, (i0, n) in enumerate(chunk):
                    a = PAD0
                    b = STRIDE * n + PAD0 - (PAD0 if d > 0 else 0)
                    nc.tensor.matmul(
                        pts[q][:, a:b],
                        wt[:, H * j : H * (j + 1)],
                        xts[(ci * qg + q) % xt_bufs][:, a + d : b + d],
                        start=(idx == 0),
                        stop=(idx == 3),
                    )

            for q, (i0, n) in enumerate(chunk):
                g = ci * qg + q
                pt = pts[q]
                ot = opool.tile([H, n * W], f32, tag="ot", name="ot")
                psrc = pt[:, 0 : STRIDE * n].rearrange("p (k c) -> p k c", c=STRIDE)
                odst = ot.rearrange("p (k c) -> p k c", c=W)
                if g % 2 == 0:
                    nc.vector.tensor_copy(odst, psrc[:, :, PAD0 : PAD0 + W])
                else:
                    nc.scalar.copy(odst, psrc[:, :, PAD0 : PAD0 + W])
                osrc = ot.rearrange("p (k c) -> p k c", c=W)
                dma_engines[(g + 1) % len(dma_engines)].dma_start(
                    y_d[i0 : i0 + n].rearrange("n h w -> h n w"), osrc
                )

    nc.compile()
    return nc


def _get_program(n_images: int):
    if n_images not in _PROGRAM_CACHE:
        _PROGRAM_CACHE[n_images] = build_program(n_images)
    return _PROGRAM_CACHE[n_images]


def kernel(x: np.ndarray, kernel: np.ndarray, _trace: bool = False):
    x = np.ascontiguousarray(x, dtype=np.float32)
    assert x.shape == (B, C, H, W), x.shape
    bands = _band_matrices(kernel)

    n_total = B * C
    n_per_core = n_total // N_CORES
    xf = x.reshape(n_total, H, W)

    nc = _get_program(n_per_core)
    in_maps = [
        {
            "x": _round_fp32r(xf[c * n_per_core : (c + 1) * n_per_core]),
            "bands": _round_fp32r(bands),
            "zeros": np.zeros((H, 2 * GROUP + 2), dtype=np.float32),
        }
        for c in range(N_CORES)
    ]
    res = bass_utils.run_bass_kernel_spmd(
        nc, in_maps, core_ids=list(range(N_CORES)), trace=_trace
    )
    y = np.concatenate([r["y"] for r in res.results], axis=0).reshape(B, C, H, W)
    if _trace:
        return y, res
    return y



# revision 2
# speedup vs baseline: 1.6550x; 1.6550x over previous
"""Trainium2 Bass kernel for nn_Blur: depthwise 4x4 FIR blur (upfirdn2d pad=(2,1)).

Full inputs in, full output out. Internally shards the 4096 (b,c) images
across 8 NeuronCores (pure data parallel, no collectives).

v2 (bf16): tolerance is rel_err < 2e-2, so all device I/O is bf16 (host-side
RNE cast) — halves HBM traffic vs fp32 (memory-regime roofline win).  The
host also pre-arranges x into the exact SBUF layout the kernel wants
([h, img-major cols] with 2 zero gap columns per image), so every DMA is a
single large per-partition-contiguous transfer (~1.6 MB, 12.5 KB/partition
descriptors) instead of many 512 B-chunk strided ones.

Compute per core (512 images of [H=128, W=128]): the 4x4 depthwise conv
factors into 4 column-convolutions along H, each a banded matmul with the
contraction over the partition (H) axis, with the W-shift (j-2) realized as
a shifted moving-operand read of an accumulating matmul:
  psum[:, c] += W_j^T @ x[:, c + (j-2)]     W_j[hi, ho] = wf[hi-ho+2, j]
Images are packed at stride 130 (2 zero gap cols) so shifted reads pick up
zero padding at image edges.  Groups of 3 images share a PSUM bank; 4
groups (4 banks) form one chunk tile so PSUM->SBUF evacuation is one big
Vector/Scalar copy per chunk.
"""

import os
import sys
from contextlib import ExitStack

for _p in ("/opt/trn_rl_repo", "/root/.axon_site/_ro/trn_rl_repo"):
    if os.path.isdir(_p) and _p not in sys.path:
        sys.path.append(_p)

import ml_dtypes
import numpy as np

import concourse.bass as bass  # noqa: F401  (engine types referenced via nc)
import concourse.tile as tile
from concourse import bacc, bass_utils, mybir

BF16 = np.dtype(ml_dtypes.bfloat16)

B, C, H, W = 16, 256, 128, 128
N_CORES = 8
GROUP = 3          # images per PSUM bank / matmul group
STRIDE = 130       # 2-col gap + 128 data cols per image in the packed layout
PAD0 = 2           # upfirdn2d pad before (both spatial dims)
TILE = 48          # images per DMA tile (must be multiple of GROUP)
QG = 4             # PSUM banks (groups) per chunk tile

_PROGRAM_CACHE: dict[int, object] = {}


def _band_matrices(kern: np.ndarray) -> np.ndarray:
    """bands[j][hi, ho] = wf[hi-ho+2, j], wf = flip(kern). Shape [4,128,128]."""
    wf = np.flip(np.asarray(kern, dtype=np.float64), (0, 1))
    bands = np.zeros((4, H, H), dtype=np.float64)
    ho = np.arange(H)
    for j in range(4):
        for i in range(4):
            d = i - PAD0            # hi - ho
            hi = ho + d
            m = (hi >= 0) & (hi < H)
            bands[j][hi[m], ho[m]] = wf[i, j]
    return np.ascontiguousarray(bands.astype(np.float32))


def _tiles(n_images: int):
    """Split n_images into DMA tiles of at most TILE images."""
    out = []
    i = 0
    while i < n_images:
        n = min(TILE, n_images - i)
        out.append((i, n))
        i += n
    return out


def _groups(n_images: int):
    """Split a tile's images into matmul groups of at most GROUP, avoiding a
    trailing 1-image group (rebalance 3+1 -> 2+2)."""
    out = []
    i = 0
    while i < n_images:
        n = min(GROUP, n_images - i)
        out.append((i, n))
        i += n
    if len(out) >= 2 and out[-1][1] == 1:
        i0, n0 = out[-2]
        out[-2] = (i0, 2)
        out[-1] = (i0 + 2, 2)
    return out


def build_program(n_images: int, xt_bufs: int = 3):
    """Build + compile the per-core Bass program for n_images [128,128] images.

    DRAM layout (host-prepared, bf16):
      x: [H, n_images*STRIDE + 2]  image k's column w at STRIDE*k + 2 + w,
         cols {STRIDE*k, STRIDE*k+1} and the trailing 2 are zeros.
      y: [H, n_images*W]           image k's column w at W*k + w.
    """
    nc = bacc.Bacc("TRN2", target_bir_lowering=False, debug=False)
    f32 = mybir.dt.float32
    bf16 = mybir.dt.bfloat16

    x_d = nc.dram_tensor("x", [H, n_images * STRIDE + 2], bf16, kind="ExternalInput")
    b_d = nc.dram_tensor("bands", [4, H, H], bf16, kind="ExternalInput")
    y_d = nc.dram_tensor("y", [H, n_images * W], bf16, kind="ExternalOutput")

    tiles = _tiles(n_images)

    with ExitStack() as ctx:
        tc = ctx.enter_context(tile.TileContext(nc))
        wpool = ctx.enter_context(tc.tile_pool(name="wpool", bufs=1))
        xpool = ctx.enter_context(tc.tile_pool(name="xpool", bufs=xt_bufs))
        opool = ctx.enter_context(tc.tile_pool(name="opool", bufs=xt_bufs))
        ppool = ctx.enter_context(tc.tile_pool(name="ppool", bufs=2, space="PSUM"))

        wt = wpool.tile([H, 4 * H], bf16)
        nc.sync.dma_start(
            wt.rearrange("p (j b) -> p j b", b=H), b_d.rearrange("j a b -> a j b")
        )

        xts: dict[int, object] = {}

        def emit_in_dma(ti):
            i0, tn = tiles[ti]
            xt = xpool.tile([H, tn * STRIDE + 2], bf16, tag="xt", name="xt")
            nc.sync.dma_start(
                xt, x_d[:, i0 * STRIDE : i0 * STRIDE + tn * STRIDE + 2]
            )
            xts[ti] = xt

        emit_in_dma(0)
        if len(tiles) > 1:
            emit_in_dma(1)

        copy_idx = 0
        for ti, (i0, tn) in enumerate(tiles):
            if ti + 2 < len(tiles):
                emit_in_dma(ti + 2)
            xt = xts.pop(ti)

            gs = _groups(tn)
            chunks = [gs[s : s + QG] for s in range(0, len(gs), QG)]
            ot = opool.tile([H, tn * W], bf16, tag="ot", name="ot")

            for chunk in chunks:
                nq = len(chunk)
                pt = ppool.tile([H, 512 * nq], f32, tag="pt", name="pt")
                # j-outer order amortizes the 4 stationary (band) loads over
                # the whole chunk; j=2 (d=0) first for the full-width
                # has_written-clearing write.
                for idx, j in enumerate((2, 0, 1, 3)):
                    d = j - PAD0
                    for q, (goff, n) in enumerate(chunk):
                        a = PAD0
                        b = STRIDE * n + PAD0 - (PAD0 if d > 0 else 0)
                        base = goff * STRIDE
                        nc.tensor.matmul(
                            pt[:, 512 * q + a : 512 * q + b],
                            wt[:, H * j : H * (j + 1)],
                            xt[:, base + a + d : base + b + d],
                            start=(idx == 0),
                            stop=(idx == 3),
                        )

                # PSUM -> SBUF evacuation (fp32 -> bf16).  One strided copy
                # per chunk when the chunk is uniform (all groups GROUP-sized);
                # per-group copies otherwise (ragged tail).
                uniform = all(n == GROUP for _, n in chunk)
                eng = (nc.vector, nc.scalar)
                if uniform:
                    psrc = (
                        pt.rearrange("p (q c) -> p q c", c=512)[
                            :, :, : GROUP * STRIDE
                        ]
                        .rearrange("p q (k c) -> p q k c", c=STRIDE)[
                            :, :, :, PAD0 : PAD0 + W
                        ]
                    )
                    odst = ot[
                        :, chunk[0][0] * W : (chunk[-1][0] + GROUP) * W
                    ].rearrange("p (q k c) -> p q k c", q=nq, c=W)
                    e = eng[copy_idx % 2]
                    if e is nc.vector:
                        e.tensor_copy(odst, psrc)
                    else:
                        e.copy(odst, psrc)
                    copy_idx += 1
                else:
                    for q, (goff, n) in enumerate(chunk):
                        psrc = pt[:, 512 * q : 512 * q + STRIDE * n].rearrange(
                            "p (k c) -> p k c", c=STRIDE
                        )[:, :, PAD0 : PAD0 + W]
                        odst = ot[:, goff * W : (goff + n) * W].rearrange(
                            "p (k c) -> p k c", c=W
                        )
                        e = eng[copy_idx % 2]
                        if e is nc.vector:
                            e.tensor_copy(odst, psrc)
                        else:
                            e.copy(odst, psrc)
                        copy_idx += 1

            nc.gpsimd.dma_start(y_d[:, i0 * W : (i0 + tn) * W], ot)

    nc.compile()
    return nc


def _get_program(n_images: int):
    if n_images not in _PROGRAM_CACHE:
        _PROGRAM_CACHE[n_images] = build_program(n_images)
    return _PROGRAM_CACHE[n_images]


def _pack_input(xc_bf16: np.ndarray) -> np.ndarray:
    """[n, H, W] bf16 -> [H, n*STRIDE + 2] bf16 gap layout."""
    n = xc_bf16.shape[0]
    arr = np.zeros((H, n * STRIDE + 2), dtype=BF16)
    v = np.lib.stride_tricks.as_strided(
        arr,
        shape=(H, n, STRIDE),
        strides=(arr.strides[0], STRIDE * arr.itemsize, arr.itemsize),
    )
    v[:, :, PAD0:] = xc_bf16.transpose(1, 0, 2)
    return arr


def kernel(x: np.ndarray, kernel: np.ndarray, _trace: bool = False):
    x = np.ascontiguousarray(x, dtype=np.float32)
    assert x.shape == (B, C, H, W), x.shape
    bands_bf = _band_matrices(kernel).astype(BF16)

    n_total = B * C
    n_per_core = n_total // N_CORES
    xb = x.reshape(n_total, H, W).astype(BF16)

    nc = _get_program(n_per_core)
    in_maps = [
        {
            "x": _pack_input(xb[c * n_per_core : (c + 1) * n_per_core]),
            "bands": bands_bf,
        }
        for c in range(N_CORES)
    ]
    res = bass_utils.run_bass_kernel_spmd(
        nc, in_maps, core_ids=list(range(N_CORES)), trace=_trace
    )
    y = np.empty((n_total, H, W), dtype=np.float32)
    for c, r in enumerate(res.results):
        yc = np.asarray(r["y"]).reshape(H, n_per_core, W)
        y[c * n_per_core : (c + 1) * n_per_core] = yc.transpose(1, 0, 2).astype(
            np.float32
        )
    y = y.reshape(B, C, H, W)
    if _trace:
        return y, res
    return y
